# revision 42
# baseline (speedup 1.0000x reference)
"""AbLang2 transformer encoder layer on 8 Trainium2 NeuronCores.

Sharding: data-parallel over batch B=8 -> one batch element per core.
Per-core dataflow (all on-chip, single TileContext):
  x [1024,768] -> LN1 (stats natural, affine fused into the transpose
  evacuation) -> hT -> qT,kT (+RoPE), v(natural)
  -> per-head: S^T = K @ Q^T, E = exp(S^T + mask), O^T_aug = [V|1]^T @ E
  -> 1/s via on-chip reciprocal + K=1 f32r broadcast matmul
  -> out-proj + residual -> r -> LN2 -> h2T -> FFN (two dff halves,
  weight slots reused) -> y
Weights are shipped pre-transposed fp32 (same HBM bytes as the original
layout) and cast to bf16 by gpsimd DMAs on load.  All matmuls run in
bf16 with fp32 PSUM accumulation.
"""

from contextlib import ExitStack

import numpy as np

import concourse.bass as bass
import concourse.tile as tile
from concourse import bacc, mybir
from concourse.bass_utils import run_bass_kernel_spmd
from concourse.masks import make_identity

F32 = mybir.dt.float32
F32R = mybir.dt.float32r
BF16 = mybir.dt.bfloat16

D = 768
H = 12
HD = 64
FF = 3072
B = 8
N = 1024
P = 128
NT = N // P   # 8 token tiles
DT = D // P   # 6 d_model tiles
FT = FF // P  # 24 ffn tiles
EPS = 1e-5

last_result = None  # BassKernelResults from the most recent run (for test.py)


def _bf16(a):
    import ml_dtypes
    return np.asarray(a, dtype=np.float32).astype(ml_dtypes.bfloat16)


def _build_kernel():
    nc = bacc.Bacc("TRN2", target_bir_lowering=False, debug=False)

    dram = {}

    def din(name, shape, dtype=F32):
        dram[name] = nc.dram_tensor(name, list(shape), dtype, kind="ExternalInput").ap()
        return dram[name]

    din("x", (N, D))
    din("mask01", (P, NT))              # multiplicative key mask, tiled
    din("wqT", (D, D))                  # W.T, f32; wq pre-scaled by 1/64
    din("wkT", (D, D))
    din("wvT", (D, D))
    din("woT", (D, D))
    din("w1T", (D, FF))
    din("w2T", (FF, D))
    din("bq", (P, DT))                  # biases tiled per-partition [p, m]
    din("bk", (P, DT))
    din("b1", (P, FT))
    din("ln1_wt", (P, DT))              # LN affine, tiled [p, d_tile]
    din("ln1_bt", (P, DT))
    din("ln2_wt", (P, DT))
    din("ln2_bt", (P, DT))
    din("x2", (N, D))                   # x + bo2 (for the phase-5 residual)
    din("b2_b", (P, D), BF16)           # host pre-broadcast constant
    din("cos2", (P, N), BF16)           # cos table, 2 heads stacked
    din("sin2", (P, N), BF16)
    din("rt", (P, P), BF16)             # lhsT of rotate_half permutation

    y_d = nc.dram_tensor("y", [N, D], F32, kind="ExternalOutput").ap()

    with tile.TileContext(nc) as tc:
        with ExitStack() as ctx:
            _body(ctx, tc, dram, y_d)
    nc.compile()
    return nc


def _body(ctx, tc, dram, y_d):
    nc = tc.nc

    # ------------- pools -------------
    consts = ctx.enter_context(tc.tile_pool(name="consts", bufs=1))
    big = ctx.enter_context(tc.tile_pool(name="big", bufs=1))
    wts = ctx.enter_context(tc.tile_pool(name="wts", bufs=1))    # 24 x [P,768] bf16
    w1p = ctx.enter_context(tc.tile_pool(name="w1p", bufs=1))    # 6 x [P,1536] bf16
    xpool = ctx.enter_context(tc.tile_pool(name="xpool", bufs=3))
    hpool = ctx.enter_context(tc.tile_pool(name="hpool", bufs=1))
    vpool = ctx.enter_context(tc.tile_pool(name="vpool", bufs=1))
    invb = ctx.enter_context(tc.tile_pool(name="invb", bufs=1))
    rpool = ctx.enter_context(tc.tile_pool(name="rpool", bufs=1))
    ypool = ctx.enter_context(tc.tile_pool(name="ypool", bufs=1))
    small = ctx.enter_context(tc.tile_pool(name="small", bufs=3))

    ps_mm = ctx.enter_context(tc.tile_pool(name="ps_mm", bufs=2, space="PSUM"))
    ps_b = ctx.enter_context(tc.tile_pool(name="ps_b", bufs=2, space="PSUM"))

    def big_tile(tag_i, dtype=BF16, name=None):
        return big.tile([P, N], dtype, tag=f"b{tag_i}", name=name or f"bt{tag_i}")

    def wt_tile(tag_i, name):
        return wts.tile([P, D], BF16, tag=f"w{tag_i}", name=name)

    # big-pool tag map:
    #   b0-b5   qropeT   (reused by gT 0-5 in FFN halves)
    #   b6-b11  kropeT   (reused by gT 6-11)
    #   b12-b13 pre-rope qT/kT staging
    #   b14-b16 exp(S.T) stream
    #   b17-b22 O.T bf16 (normalized in place)
    #   b23-b28 hT / h2T

    # ------------- constants -------------
    ident = consts.tile([P, P], BF16)
    make_identity(nc, ident)
    ones_f = consts.tile([P, P], F32)
    nc.vector.memset(ones_f, 1.0)
    ones128 = consts.tile([P, P], F32R)
    with nc.allow_low_precision(reason="f32r ones for 1/s broadcast"):
        nc.vector.tensor_copy(out=ones128, in_=ones_f)
    eps_t = consts.tile([P, 1], F32)
    nc.vector.memset(eps_t, EPS)

    def _load(nm, shape, dtype):
        t = consts.tile(list(shape), dtype, name=nm + "_sb")
        nc.sync.dma_start(out=t, in_=dram[nm])
        return t

    rt_sb = _load("rt", (P, P), BF16)
    cos2 = _load("cos2", (P, N), BF16)
    sin2 = _load("sin2", (P, N), BF16)
    mask_sb = _load("mask01", (P, NT), F32)
    bq_sb = _load("bq", (P, DT), F32)
    bk_sb = _load("bk", (P, DT), F32)
    b1_sb = _load("b1", (P, FT), F32)
    ln1_wt = _load("ln1_wt", (P, DT), F32)
    ln1_bt = _load("ln1_bt", (P, DT), F32)
    ln2_wt = _load("ln2_wt", (P, DT), F32)
    ln2_bt = _load("ln2_bt", (P, DT), F32)
    b2_b = _load("b2_b", (P, D), BF16)

    # ------------- fused LN + transpose -------------
    def layer_norm_T(src_tiles, w_t, b_t, label):
        """LN stats per 128-token tile (natural layout), centered/scaled
        tiles transposed on PE, LN affine applied during the PSUM
        evacuation.  Returns DT transposed tiles [P, N] bf16 (tags b23+)."""
        t1s = []
        for t in range(NT):
            xt = src_tiles[t]
            stats = small.tile([P, 3, 6], F32, tag="stats", name=f"st_{label}{t}")
            for g in range(3):
                nc.vector.bn_stats(out=stats[:, g, :], in_=xt[:, g * 256:(g + 1) * 256])
            mv = small.tile([P, 2], F32, tag="mv", name=f"mv_{label}{t}")
            nc.vector.bn_aggr(out=mv, in_=stats)
            rstd = small.tile([P, 1], F32, tag="rstd", name=f"rs_{label}{t}")
            nc.scalar.activation(out=rstd, in_=mv[:, 1:2],
                                 func=mybir.ActivationFunctionType.Sqrt,
                                 bias=eps_t, scale=1.0)
            nc.vector.reciprocal(out=rstd, in_=rstd)
            nmu = small.tile([P, 1], F32, tag="nmu", name=f"nmu_{label}{t}")
            nc.vector.tensor_scalar(out=nmu, in0=mv[:, 0:1], scalar1=rstd,
                                    scalar2=-1.0, op0=mybir.AluOpType.mult,
                                    op1=mybir.AluOpType.mult)
            t1 = hpool.tile([P, D], BF16, tag=f"t1_{t}", name=f"t1_{label}{t}")
            nc.scalar.activation(out=t1, in_=xt,
                                 func=mybir.ActivationFunctionType.Identity,
                                 bias=nmu, scale=rstd)
            t1s.append(t1)
        dst = []
        for d in range(DT):
            pt = ps_b.tile([P, N], BF16, tag="pv", name=f"pt_{label}{d}")
            for m in range(NT):
                nc.tensor.transpose(pt[:, m * P:(m + 1) * P],
                                    t1s[m][:, d * P:(d + 1) * P], ident)
            o = big_tile(23 + d, name=f"{label}{d}")
            nc.vector.tensor_scalar(out=o, in0=pt, scalar1=w_t[:, d:d + 1],
                                    scalar2=b_t[:, d:d + 1],
                                    op0=mybir.AluOpType.mult,
                                    op1=mybir.AluOpType.add)
            dst.append(o)
        return dst

    # ---------------- phase 0: x + LN1 + hT ----------------
    x_tiles = []
    for t in range(NT):
        xt = xpool.tile([P, D], F32, tag="x", name=f"x{t}")
        nc.sync.dma_start(out=xt, in_=dram["x"][t * P:(t + 1) * P, :])
        x_tiles.append(xt)
    hT = layer_norm_T(x_tiles, ln1_wt, ln1_bt, "hT")

    # ---------------- phase 1: attention weights ----------------
    def load_wt(dname, base_tag, label):
        tiles = []
        for k in range(DT):
            t = wt_tile(base_tag + k, f"{label}{k}")
            nc.gpsimd.dma_start(out=t, in_=dram[dname][k * P:(k + 1) * P, :])
            tiles.append(t)
        return tiles

    wqT = load_wt("wqT", 0, "wqT")
    wkT = load_wt("wkT", 6, "wkT")
    wvT = load_wt("wvT", 12, "wvT")
    woT = load_wt("woT", 18, "woT")

    # ---------------- phase 2: qT, kT + RoPE ----------------
    def proj_rope_m(wT, bias_sb, out_base, label, m):
        if True:
            ps = ps_mm.tile([P, N], F32, tag="mm", name=f"ps_{label}{m}")
            for k in range(DT):
                for j in range(2):
                    nc.tensor.matmul(
                        ps[:, j * 512:(j + 1) * 512],
                        wT[k][:, m * P:(m + 1) * P],
                        hT[k][:, j * 512:(j + 1) * 512],
                        start=(k == 0), stop=(k == DT - 1))
            sb = big_tile(12 + (m % 2), name=f"{label}pre{m}")
            nc.scalar.activation(out=sb, in_=ps,
                                 func=mybir.ActivationFunctionType.Identity,
                                 bias=bias_sb[:, m:m + 1], scale=1.0)
            rp = ps_b.tile([P, N], F32, tag="pv", name=f"psr_{label}{m}")
            for j in range(2):
                nc.tensor.matmul(rp[:, j * 512:(j + 1) * 512], rt_sb,
                                 sb[:, j * 512:(j + 1) * 512],
                                 start=True, stop=True)
            a = small.tile([P, N], BF16, tag="ropea", name=f"ra_{label}{m}")
            nc.vector.tensor_mul(out=a, in0=sb, in1=cos2)
            b = small.tile([P, N], BF16, tag="ropeb", name=f"rb_{label}{m}")
            nc.vector.tensor_mul(out=b, in0=rp, in1=sin2)
            o = big_tile(out_base + m, name=f"{label}r{m}")
            nc.vector.tensor_add(out=o, in0=a, in1=b)
            return o

    def proj_v(m):
        ps = ps_mm.tile([P, N], F32, tag="mm", name=f"ps_v{m}")
        for k in range(DT):
            for n0, nn in ((0, 512), (512, 256)):
                nc.tensor.matmul(ps[:, n0:n0 + nn],
                                 hT[k][:, m * P:(m + 1) * P],
                                 wvT[k][:, n0:n0 + nn],
                                 start=(k == 0), stop=(k == DT - 1))
        va = vpool.tile([P, H, HD + 1], BF16, tag=f"v{m}", name=f"v{m}")
        nc.vector.memset(va, 1.0)
        nc.vector.tensor_scalar_mul(out=va[:, :, HD:HD + 1],
                                    in0=va[:, :, HD:HD + 1],
                                    scalar1=mask_sb[:, m:m + 1])
        nc.vector.tensor_scalar(
            out=va[:, :, 0:HD],
            in0=ps[:, 0:D].rearrange("p (h d) -> p h d", h=H),
            scalar1=mask_sb[:, m:m + 1], scalar2=None,
            op0=mybir.AluOpType.mult)
        return va

    # ---------------- phase 4: attention ----------------
    ot_n = [big_tile(17 + i, name=f"ot{i}") for i in range(DT)]
    qT, kT = [None] * DT, [None] * DT

    def attend_pair(t):
        for hh in range(2):
            h = 2 * t + hh
            half = hh * HD
            ops = ps_b.tile([HD + 1, N], F32, tag="pv", name=f"pv{h}")
            for m in range(NT):
                ps = ps_mm.tile([P, N], F32, tag="mm", name=f"ps_s{h}_{m}")
                for j in range(2):
                    nc.tensor.matmul(
                        ps[:, j * 512:(j + 1) * 512],
                        kT[t][half:half + HD, m * P:(m + 1) * P],
                        qT[t][half:half + HD, j * 512:(j + 1) * 512],
                        start=True, stop=True)
                et = big_tile(29 + ((h * NT + m) % 8), name=f"et{h}_{m}")
                nc.scalar.activation(out=et, in_=ps,
                                     func=mybir.ActivationFunctionType.Exp)
                for j in range(2):
                    nc.tensor.matmul(
                        ops[:, j * 512:(j + 1) * 512],
                        v_aug[m][:, h, :],
                        et[:, j * 512:(j + 1) * 512],
                        start=(m == 0), stop=(m == NT - 1))
            nc.vector.tensor_copy(out=ot_n[t][half:half + HD, :], in_=ops[0:HD, :])
            # 1/s on-chip: reciprocal straight from PSUM at partition 64
            # (32-aligned base), broadcast via a K=1 f32r matmul, then
            # scale O.T in place.  Nothing lands on the ACT engine.
            sc = invb.tile([P, N], F32R, tag="stmp", name=f"sc{h}")
            with nc.allow_low_precision(reason="1/s broadcast via f32r matmul"):
                nc.vector.reciprocal(out=sc[HD:HD + 1, :], in_=ops[HD:HD + 1, :])
            psb = ps_b.tile([P, N], F32, tag="pv", name=f"psb{h}")
            for j in range(2):
                nc.tensor.matmul(
                    psb[:, j * 512:(j + 1) * 512],
                    ones128[HD:HD + 1, :],
                    sc[HD:HD + 1, j * 512:(j + 1) * 512],
                    start=True, stop=True)
            nc.vector.tensor_mul(out=ot_n[t][half:half + HD, :],
                                 in0=ot_n[t][half:half + HD, :],
                                 in1=psb[half:half + HD, :])

    for t in range(DT):
        qT[t] = proj_rope_m(wqT, bq_sb, 0, "q", t)
    for t in range(DT):
        kT[t] = proj_rope_m(wkT, bk_sb, 6, "k", t)
    v_aug = [proj_v(m) for m in range(NT)]
    for t in range(DT):
        attend_pair(t)

    # ---------------- phase 5: out-proj + residual ----------------
    r_tiles = []
    for m in range(NT):
        ps = ps_mm.tile([P, N], F32, tag="mm", name=f"ps_o{m}")
        for k in range(DT):
            for n0, nn in ((0, 512), (512, 256)):
                nc.tensor.matmul(ps[:, n0:n0 + nn],
                                 ot_n[k][:, m * P:(m + 1) * P],
                                 woT[k][:, n0:n0 + nn],
                                 start=(k == 0), stop=(k == DT - 1))
        xr = xpool.tile([P, D], F32, tag="x", name=f"xr{m}")
        nc.sync.dma_start(out=xr, in_=dram["x2"][m * P:(m + 1) * P, :])
        rt = rpool.tile([P, D], F32, tag=f"r{m}", name=f"r{m}")
        nc.vector.tensor_add(out=rt, in0=ps[:, 0:D], in1=xr)
        r_tiles.append(rt)

    # ---------------- phase 6: LN2 + FFN (two dff halves) ----------------
    h2T = layer_norm_T(r_tiles, ln2_wt, ln2_bt, "h2T")

    FH = FT // 2  # 12 dff tiles per half
    for half_i in range(2):
        f0 = half_i * FH
        # w1T half -> w1p combined tiles [P, 1536]; w2T half -> tags
        # w12..w23 (reuse wv/wo).  Slot WARs alone order the loads.
        w1T = []
        for k in range(DT):
            t = w1p.tile([P, 2 * D], BF16, tag=f"w1_{k}", name=f"w1T_{half_i}_{k}")
            nc.gpsimd.dma_start(
                out=t, in_=dram["w1T"][k * P:(k + 1) * P, f0 * P:f0 * P + 2 * D])
            w1T.append(t)
        w2T = []
        for f in range(FH):
            t = wt_tile(12 + f, f"w2T_{half_i}_{f}")
            nc.gpsimd.dma_start(
                out=t, in_=dram["w2T"][(f0 + f) * P:(f0 + f + 1) * P, :])
            w2T.append(t)

        gT = []
        for mi in range(FH):
            m = f0 + mi
            ps = ps_mm.tile([P, N], F32, tag="mm", name=f"ps_g{m}")
            for k in range(DT):
                for j in range(2):
                    nc.tensor.matmul(
                        ps[:, j * 512:(j + 1) * 512],
                        w1T[k][:, mi * P:(mi + 1) * P],
                        h2T[k][:, j * 512:(j + 1) * 512],
                        start=(k == 0), stop=(k == DT - 1))
            gt = big_tile(mi, name=f"g{m}")
            nc.scalar.activation(out=gt, in_=ps,
                                 func=mybir.ActivationFunctionType.Gelu,
                                 bias=b1_sb[:, m:m + 1], scale=1.0)
            gT.append(gt)

        for m in range(NT):
            ps = ps_mm.tile([P, N], F32, tag="mm", name=f"ps_f{half_i}_{m}")
            for k in range(FH):
                for n0, nn in ((0, 512), (512, 256)):
                    nc.tensor.matmul(ps[:, n0:n0 + nn],
                                     gT[k][:, m * P:(m + 1) * P],
                                     w2T[k][:, n0:n0 + nn],
                                     start=(k == 0), stop=(k == FH - 1))
            nc.vector.tensor_add(out=r_tiles[m], in0=r_tiles[m], in1=ps[:, 0:D])

    for m in range(NT):
        yt = ypool.tile([P, D], F32, tag="y", name=f"y{m}")
        nc.vector.tensor_add(out=yt, in0=r_tiles[m], in1=b2_b)
        nc.sync.dma_start(out=y_d[m * P:(m + 1) * P, :], in_=yt)


def _host_prep(inputs):
    """Build per-core input maps from the full problem inputs."""
    g = {k: np.asarray(v) for k, v in inputs.items()}
    x = g["x"].astype(np.float32)
    pm = np.asarray(g["padding_mask"]).astype(bool)
    freqs = g["freqs"].astype(np.float32)

    scale = 1.0 / 64.0  # SCALING (HD^-0.5=1/8) * 1/sqrt(HD) (=1/8)
    wq_s = (g["wq"] * scale).astype(np.float32)
    bq_s = (g["bq"] * scale).astype(np.float32)

    ang = np.outer(np.arange(N, dtype=np.float32), freqs)       # [N, 32]
    ang = np.repeat(ang, 2, axis=-1)                            # [N, 64]
    cosT = np.cos(ang).T                                        # [64, N]
    sinT = np.sin(ang).T
    cos2 = _bf16(np.concatenate([cosT, cosT], axis=0))          # [128, N]
    sin2 = _bf16(np.concatenate([sinT, sinT], axis=0))

    rt = np.zeros((P, P), np.float32)
    for i in range(P // 2):
        rt[2 * i + 1, 2 * i] = -1.0   # lhsT[c,r] = R[r,c]; R[2i,2i+1] = -1
        rt[2 * i, 2 * i + 1] = 1.0    # R[2i+1,2i] = +1
    rt = _bf16(rt)

    bo2 = (g["bo"] + g["wo"] @ g["bv"]).astype(np.float32)

    def tile_bias(b, nt):
        return np.ascontiguousarray(b.astype(np.float32).reshape(nt, P).T)

    shared = dict(
        wqT=np.ascontiguousarray(wq_s.T),
        wkT=np.ascontiguousarray(g["wk"].astype(np.float32).T),
        wvT=np.ascontiguousarray(g["wv"].astype(np.float32).T),
        woT=np.ascontiguousarray(g["wo"].astype(np.float32).T),
        w1T=np.ascontiguousarray(g["w1"].astype(np.float32).T),
        w2T=np.ascontiguousarray(g["w2"].astype(np.float32).T),
        bq=tile_bias(bq_s, DT), bk=tile_bias(g["bk"], DT),
        b1=tile_bias(g["b1"], FT),
        ln1_wt=tile_bias(g["ln1_w"], DT), ln1_bt=tile_bias(g["ln1_b"], DT),
        ln2_wt=tile_bias(g["ln2_w"], DT), ln2_bt=tile_bias(g["ln2_b"], DT),
        b2_b=_bf16(np.broadcast_to(g["b2"].astype(np.float32), (P, D))),
        cos2=cos2, sin2=sin2, rt=rt,
    )

    in_maps = []
    for b in range(B):
        mb = np.where(pm[b], 0.0, 1.0).astype(np.float32)       # [N]
        per = dict(shared)
        per["x"] = np.ascontiguousarray(x[b])
        per["x2"] = np.ascontiguousarray(x[b] + bo2)
        per["mask01"] = np.ascontiguousarray(mb.reshape(NT, P).T)
        in_maps.append(per)
    return in_maps


_nc_cache = None


def kernel(**inputs):
    global _nc_cache, last_result
    if _nc_cache is None:
        _nc_cache = _build_kernel()
    in_maps = _host_prep(inputs)
    res = run_bass_kernel_spmd(_nc_cache, in_maps, list(range(B)))
    last_result = res
    y = np.stack([np.asarray(res.results[b]["y"]) for b in range(B)], axis=0)
    return y.astype(np.float32)
